# revision 9
# baseline (speedup 1.0000x reference)
"""Batch-hard triplet loss on 8 Trainium2 NeuronCores (Bass/Tile).

Math (reference): L2-normalize rows of embeddings [4096, 512]; gram = e@e.T;
dist = sqrt(clip(2 - 2*gram, 0)); per row: hardest positive = max dist over
same-label (excl. self), hardest negative = min dist over different-label;
loss = mean over valid rows of relu(d_ap - d_an + margin).  Since dist is
monotone-decreasing in gram, both row reductions are done on gram.

Kernel design (each core computes a [512, 4096] block of the gram):

- Host prep (loss is permutation invariant): rows sorted by label,
  normalized in fp32, transposed, quantized to fp8 e4m3 (loss rel err
  ~2e-4, threshold 2e-2). Masking is folded into the matmul: 128 one-hot
  class rows scaled -2 on the rhs x +2*onehot(own label) channels on the
  lhs make the PE compute ghat = gram - 4*same. Positives (incl. self)
  land in [-5,-3], negatives in (-1,1), so max ghat = hardest-negative
  gram and min ghat + 4 = hardest-positive gram; validity = (pmin < -3.1)
  & (nmax > -1.5) reproduces the reference's row filtering.
- Per-core column ROTATION by (512c - 64) mod 4096 puts each core's own
  rows at columns [64, 576): the program is identical on all cores (pure
  SPMD via input data), every same-class pair sits in rotated columns
  [0, 640), so (a) the one-hot pair is only applied to the first 2 of 8
  column slabs, and (b) the hardest-positive is a fixed 256-wide windowed
  min over the SAME PSUM tiles the full-pass max uses — no separate
  masked matmul pass.
- Matmuls run fp8 perf_mode=DoubleRow: operands are 3D APs [128, 2, X]
  carrying two contraction rows per partition (256-row contraction per
  instruction): 4 row tiles x 8 slabs x (3 pairs for slabs 0-1, 2 after)
  = 72 matmuls per core.
- Row maxes are engine-split so they don't serialize on DVE: each
  (piece, row-tile) accumulates into two 2-bank PSUM strips [128, 1024]
  (four 512-col accumulation groups), of which DVE direct-reduces only
  the first slab (f32, also feeding the hardest-positive window -- the
  strip makes the m=3 window contiguous); the Activation engine copies
  the other three slabs PSUM -> fp16 SBUF (512- and 1024-wide ops), and
  per row tile one fp16 tensor_tensor max + one wide reduce emit the row
  max. Four strips pipeline in the 8 PSUM banks (m outer / piece inner).
  (Rejected routes: tensor_tensor_reduce compiles but its custom-DVE
  ucode faults at runtime over axon; two-PSUM-input TTR = NCC_IBVF027;
  TensorTensor on Pool = NCC_IXCG966; DMA max-accum = NCC_IBIR077;
  gpsimd tensor_reduce is partition-axis-only.)
- Tail: distances, validity and masked (sum, count) partials; partition
  all-reduce + output DMA run on the Pool engine so PE/SP never stall.
  Each core emits 8 bytes; the host does the final divide.

DRAM layout (host-packed): x [128, 2, 2, 2, 2048] fp8 = partition-major
embedding pairs (pair j holds k-chunks 2j, 2j+1) split in 2 column pieces;
ohm [128, 2, 1024] = (-2 one-hot for columns 0-1024, zeros); ohp
[128, 2, 512] = (+2 one-hot of own rows, zeros).

_build_program(repeat=R) unrolls the body R times (rotating tile pools,
steady-state overlap) so test.py can measure the marginal device time per
execution as a slope over R, cancelling the ~1 ms axon per-dispatch launch
overhead.  Measured marginal device time: ~20-25 us/execution across runs
(estimator noise +-2-3 us; TimelineSim model: 17.9 us, which does not
price DoubleRow weight loads; the fp16 predecessor measured 42 us; the
original kernel's printed baseline was 1578 us).  The j-outer matmul
ordering (4 consecutive matmuls share one lhsT across the 4 open
accumulation groups) measured 21.7k vs 22.2k ns j-inner -- within noise
but never worse, kept for the weight-load locality.
"""

import numpy as np

N, D, NCLS, NCORES = 4096, 512, 128, 8
R = N // NCORES
MT = R // 128
KCH = D // 128
SLABS = N // 512
WPAD = 64
PIECE = 2048
MARGIN = 0.3
PAIRS = (KCH + 1 + 1) // 2      # 3 operand pairs (incl. zero-padded one-hot)

_CACHE = {}


def _build_program(repeat=1):
    import concourse.bacc as bacc
    import concourse.tile as tile
    from concourse import mybir
    import concourse.bass_isa as bass_isa

    f32 = mybir.dt.float32
    f16 = mybir.dt.float16
    f8 = mybir.dt.float8e4
    Alu = mybir.AluOpType
    Act = mybir.ActivationFunctionType
    Ax = mybir.AxisListType
    DR = mybir.MatmulPerfMode.DoubleRow

    nc = bacc.Bacc("TRN2", target_bir_lowering=False, debug=False,
                   num_devices=NCORES)

    NP = N // PIECE          # 2 column pieces
    SPP = PIECE // 512       # 4 slabs per piece

    x_d = nc.dram_tensor("x", [128, PAIRS - 1, NP, 2, PIECE], f8,
                         kind="ExternalInput").ap()
    ohm_d = nc.dram_tensor("ohm", [NCLS, 2, 1024], f8,
                           kind="ExternalInput").ap()
    ohp_d = nc.dram_tensor("ohp", [NCLS, 2, R], f8, kind="ExternalInput").ap()
    out_d = nc.dram_tensor("out", [repeat, 2], f32, kind="ExternalOutput").ap()

    with tile.TileContext(nc) as tc:
        import contextlib
        with contextlib.ExitStack() as ctx:
            nbuf = 2 if repeat > 1 else 1
            singles = ctx.enter_context(tc.tile_pool(name="singles", bufs=1))
            big = ctx.enter_context(tc.tile_pool(name="big", bufs=nbuf))
            sm = ctx.enter_context(tc.tile_pool(name="sm", bufs=nbuf))
            scr_pool = ctx.enter_context(tc.tile_pool(name="scr", bufs=2))
            ps_pool = ctx.enter_context(
                tc.tile_pool(name="ps", bufs=4, space="PSUM"))

            b_m6 = singles.tile([128, 1], f32)
            nc.gpsimd.memset(b_m6, -6.0)
            b_p2 = singles.tile([128, 1], f32)
            nc.gpsimd.memset(b_p2, 2.0)
            b_mg = singles.tile([128, 1], f32)
            nc.gpsimd.memset(b_mg, MARGIN)

            for r in range(repeat):
                # ---- input loads (contiguous, SP queue only) ----
                ohp = big.tile([NCLS, 2, R], f8, tag="ohp")
                nc.sync.dma_start(ohp, ohp_d)
                ohm = big.tile([NCLS, 2, 1024], f8, tag="ohm")
                nc.sync.dma_start(ohm, ohm_d)
                xt = {}
                for p in range(NP):
                    for j in range(PAIRS - 1):
                        t = big.tile([128, 2, PIECE], f8, tag=f"x_{j}_{p}")
                        xt[(j, p)] = t
                        nc.sync.dma_start(t, x_d[:, j, p, :, :])

                # pmax cols: h = direct f32 reduce of slab 4h (h = 0, 1),
                #            2 = fp16 TTR of slabs 1-3 and 5-7
                pmax = sm.tile([128, MT, 3], f32, tag="pmax")
                pminw = sm.tile([128, MT], f32, tag="pminw")
                lf = {}

                def lhs(j, m):
                    if j < PAIRS - 1:
                        return xt[(j, 0)][:, :, WPAD + 128 * m:
                                          WPAD + 128 * m + 128]
                    return ohp[:, :, 128 * m:128 * m + 128]

                # ---- gram blocks + row reductions ----
                # m outer / h inner; per (h, m) two 2-bank PSUM strips
                # (slabs {0,1} and {2,3} of the piece) so 4 strips pipeline
                # in the 8 PSUM banks while Act/DVE drain earlier ones.
                for m in range(MT):
                    for h in range(NP):
                        t = scr_pool.tile([128, (SPP - 1) * 512], f16,
                                          tag=f"lf{h}_{m}",
                                          name=f"lf{h}_{m}")
                        lf[(h, m)] = t
                        # j OUTER / si inner: 4 open accumulation groups
                        # across the two strips so 4 consecutive matmuls
                        # share one lhsT -- the PE's background weight
                        # buffer can then hide the DoubleRow LDWEIGHTS
                        # (DR pays ~220-cycle loads and disables FWL).
                        pps = [ps_pool.tile([128, 1024], f32, tag="pp",
                                            name=f"pp{i}")
                               for i in range(2)]
                        npairs = [PAIRS if SPP * h + si < 2 else PAIRS - 1
                                  for si in range(SPP)]
                        for j in range(max(npairs)):
                            for half in range(2):
                                for si2 in range(2):
                                    si = 2 * half + si2
                                    if j >= npairs[si]:
                                        continue
                                    rhs = (xt[(j, h)][:, :,
                                               512 * si:512 * si + 512]
                                           if j < PAIRS - 1 else
                                           ohm[:, :, 512 * si:512 * si + 512])
                                    nc.tensor.matmul(
                                        pps[half][:, 512 * si2:512 * si2 + 512],
                                        lhs(j, m), rhs,
                                        start=(j == 0),
                                        stop=(j == npairs[si] - 1),
                                        perf_mode=DR)
                        for half in range(2):
                            pp = pps[half]
                            if half == 0:
                                nc.vector.tensor_reduce(
                                    pmax[:, m, h:h + 1], pp[:, 0:512],
                                    axis=Ax.X, op=Alu.max)
                                if h == 0:
                                    # window [128m, 128m+256) is contiguous
                                    # in the slab{0,1} strip even for m=3
                                    lo = 128 * m
                                    nc.vector.tensor_reduce(
                                        pminw[:, m:m + 1],
                                        pp[:, lo:lo + 128 + 2 * WPAD],
                                        axis=Ax.X, op=Alu.min)
                                nc.scalar.copy(t[:, 0:512], pp[:, 512:1024])
                            else:
                                nc.scalar.copy(t[:, 512:1536], pp)
                    # fold the two fp16 strips of this row tile (both SBUF)
                    tj = scr_pool.tile([128, (SPP - 1) * 512], f16,
                                       tag=f"tj{m}", name=f"tj{m}")
                    nc.vector.tensor_tensor(tj, lf[(0, m)], lf[(1, m)],
                                            op=Alu.max)
                    nc.vector.tensor_reduce(pmax[:, m, 2:3], tj,
                                            axis=Ax.X, op=Alu.max)

                # ---- tail ----
                pmin = pminw
                nmax = sm.tile([128, MT], f32, tag="nmax")
                nc.vector.tensor_reduce(nmax, pmax, axis=Ax.X, op=Alu.max)
                t1 = sm.tile([128, MT], f32, tag="t1")
                nc.scalar.activation(t1, pmin, Act.Relu, bias=b_m6,
                                     scale=-2.0)
                dap = sm.tile([128, MT], f32, tag="dap")
                nc.scalar.activation(dap, t1, Act.Sqrt)
                t2 = sm.tile([128, MT], f32, tag="t2")
                nc.scalar.activation(t2, nmax, Act.Relu, bias=b_p2,
                                     scale=-2.0)
                dan = sm.tile([128, MT], f32, tag="dan")
                nc.scalar.activation(dan, t2, Act.Sqrt)
                vp = sm.tile([128, MT], f32, tag="vp")
                nc.vector.tensor_scalar(vp, pmin, -3.1, None, Alu.is_lt)
                vn = sm.tile([128, MT], f32, tag="vn")
                nc.vector.tensor_scalar(vn, nmax, -1.5, None, Alu.is_gt)
                valid = sm.tile([128, MT], f32, tag="valid")
                nc.vector.tensor_mul(valid, vp, vn)
                diff = sm.tile([128, MT], f32, tag="diff")
                nc.vector.tensor_sub(diff, dap, dan)
                per = sm.tile([128, MT], f32, tag="per")
                nc.scalar.activation(per, diff, Act.Relu, bias=b_mg,
                                     scale=1.0)
                msk = sm.tile([128, MT], f32, tag="msk")
                nc.vector.tensor_mul(msk, per, valid)
                pk = sm.tile([128, 2], f32, tag="pk")
                nc.vector.tensor_reduce(pk[:, 0:1], msk, axis=Ax.X,
                                        op=Alu.add)
                nc.vector.tensor_reduce(pk[:, 1:2], valid, axis=Ax.X,
                                        op=Alu.add)
                pr = sm.tile([128, 2], f32, tag="pr")
                nc.gpsimd.partition_all_reduce(pr, pk, channels=128,
                                               reduce_op=bass_isa.ReduceOp.add)
                nc.gpsimd.dma_start(out_d[r:r + 1, :], pr[0:1, :])

    nc.compile()
    return nc


def _prep_inputs(embeddings, labels):
    import ml_dtypes
    f8 = ml_dtypes.float8_e4m3

    x = np.asarray(embeddings, dtype=np.float32)
    lab = np.asarray(labels).astype(np.int64)
    order = np.argsort(lab, kind="stable")
    xs = x[order]
    ls = lab[order].astype(np.int32)
    norm = np.sqrt((xs * xs).sum(1, keepdims=True))
    e = xs / np.maximum(norm, 1e-12)
    eT = np.ascontiguousarray(e.T).astype(f8)                # [512, 4096]
    oh = np.zeros((NCLS, N), dtype=f8)
    oh[ls, np.arange(N)] = -2.0
    stacked = np.concatenate([eT, oh], axis=0)               # [640, 4096]

    NP = N // PIECE
    in_maps = []
    for c in range(NCORES):
        shift = (R * c - WPAD) % N
        xr = np.concatenate([stacked[:, shift:], stacked[:, :shift]], axis=1)
        # pack embedding pairs to [128, 2, NP, 2, PIECE]
        x8 = np.empty((128, PAIRS - 1, NP, 2, PIECE), dtype=f8)
        for j in range(PAIRS - 1):
            for i in range(2):
                k = 2 * j + i
                x8[:, j, :, i, :] = xr[128 * k:128 * k + 128].reshape(
                    128, NP, PIECE)
        ohm = np.zeros((NCLS, 2, 1024), dtype=f8)
        ohm[:, 0, :] = xr[4 * 128:5 * 128, :1024]
        ohp = np.zeros((NCLS, 2, R), dtype=f8)
        ohp[ls[R * c:R * c + R], 0, np.arange(R)] = 2.0
        in_maps.append({"x": x8, "ohm": ohm, "ohp": ohp})
    return in_maps


def run(embeddings, labels, trace=False):
    from concourse.bass_utils import run_bass_kernel_spmd

    if "nc" not in _CACHE:
        _CACHE["nc"] = _build_program()
    nc = _CACHE["nc"]
    in_maps = _prep_inputs(embeddings, labels)
    res = run_bass_kernel_spmd(nc, in_maps, list(range(NCORES)), trace=trace)
    tot = np.zeros(2, dtype=np.float64)
    for c in range(NCORES):
        tot += res.results[c]["out"].reshape(-1, 2)[0].astype(np.float64)
    s, cnt = tot
    loss = np.float32(s / max(cnt, 1.0)) if cnt > 0 else np.float32(0.0)
    return np.array(loss, dtype=np.float32), res


def kernel(embeddings, labels):
    loss, _ = run(embeddings, labels)
    return loss
